# revision 19
# baseline (speedup 1.0000x reference)
"""Multi-head attention (N=2, L=2048, D=1024, H=16) on 8 NeuronCores.

Sharding: core c -> (batch n = c // 4, head group g = c % 4, 4 heads each).
Each core computes Q/K/V projections for its 4 heads, flash-style attention
(S^T = K @ Q^T per k-tile, exp on ScalarE with 1/sqrt(D) folded into the
activation scale, P^T @ V via TensorE with a ones-column appended to V to get
the softmax denominator for free), normalization, and its slice of the output
projection. Host sums the 4 partial output projections per batch and adds bo.

Matmul packing: Q-projection computes head PAIRS with a full 128-row
stationary; the output projection contracts packed head-pair oN tiles
against [128, D] Wo slices (full contraction). K-projection uses
1024-wide moving operands on the wide PSUM slots. Optional fp8e4
DoubleRow path for the Q/K projections (FP8_MODE=qk|k).
"""
import os
import sys
import types

import numpy as np
import ml_dtypes

N_BATCH = 2
L = 2048
D = 1024
H = 16
HD = 64
CORES = 8
GH = 4            # heads per core
DG = GH * HD      # 256 = projected dims per core
QB = 512          # q block
KT = L // 128     # 16 k tiles
QT = L // QB      # 4 q blocks
DC = D // 128     # 8 din chunks
FP8_MODE = os.environ.get("FP8_MODE", "0")   # "0" | "k" | "qk"
W8SCALE = 16.0    # fp8 weight prescale


def _install_ntff_hook():
    """The image's antenv stub lacks axon_hooks; shim it so trace=True works."""
    if "antenv.axon_hooks" in sys.modules:
        return
    mod = types.ModuleType("antenv.axon_hooks")
    mod._hook = None
    mod.set_axon_ntff_profile_hook = lambda h: setattr(mod, "_hook", h)
    mod.get_axon_ntff_profile_hook = lambda: mod._hook
    sys.modules["antenv.axon_hooks"] = mod
    try:
        from trn_agent_boot.trn_boot import _ntff_profile_via_ctypes
        mod._hook = _ntff_profile_via_ctypes("/opt/axon/libaxon_pjrt.so")
    except Exception:
        mod._hook = None


_install_ntff_hook()

import concourse.bacc as bacc
import concourse.mybir as mybir
import concourse.tile as tile
from concourse.bass_utils import run_bass_kernel_spmd

F32 = mybir.dt.float32
F16 = mybir.dt.float16
F8 = mybir.dt.float8e4
AF = mybir.ActivationFunctionType
MULT = mybir.AluOpType.mult
ADD = mybir.AluOpType.add
DR = mybir.MatmulPerfMode.DoubleRow

_CACHE = {}


_TABLES_PATCHED = False


def _patch_act_tables():
    """Prefer natural_log_exp_and_others so Exp and Ln share one table set."""
    global _TABLES_PATCHED
    if _TABLES_PATCHED:
        return
    import concourse.bacc as _bacc
    import concourse.hw_specs as _hw
    orig_fn = _hw.get_activation_tables

    def patched(arch):
        import concourse.mybir as _mybir
        tabs = dict(orig_fn(arch))
        pref = "natural_log_exp_and_others"
        if pref not in tabs:
            return tabs
        drop = {_mybir.ActivationFunctionType.Exp,
                _mybir.ActivationFunctionType.Ln}
        return {k: (v if k == pref else (set(v) - drop))
                for k, v in tabs.items()}

    _bacc.get_activation_tables = patched
    _TABLES_PATCHED = True


def _build(use_bias, use_mask):
    key = (use_bias, use_mask, FP8_MODE)
    if key in _CACHE:
        return _CACHE[key]
    if os.environ.get("ACT_TABLE_PATCH", "1") == "1":
        _patch_act_tables()

    nc = bacc.Bacc("TRN2", debug=False, num_devices=CORES)

    q8 = FP8_MODE == "qk"
    k8 = FP8_MODE in ("qk", "k")
    FQ = F8 if q8 else F16
    FK = F8 if k8 else F16

    xqT = nc.dram_tensor("xqT", [D, L], FQ, kind="ExternalInput").ap()
    xkT = nc.dram_tensor("xkT", [D, L], FK, kind="ExternalInput").ap()
    xvT = nc.dram_tensor("xvT", [D, L], F16, kind="ExternalInput").ap()
    aq = nc.dram_tensor("aq", [128, DC * DG], FQ, kind="ExternalInput").ap()
    ak = nc.dram_tensor("ak", [128, DC * DG], FK, kind="ExternalInput").ap()
    av = nc.dram_tensor("av", [128, DC * DG], F16, kind="ExternalInput").ap()
    bo = nc.dram_tensor("bo", [128, 2 * D], F16, kind="ExternalInput").ap()
    bq = nc.dram_tensor("bq", [1, DG], F16, kind="ExternalInput").ap()
    bk = nc.dram_tensor("bk", [1, DG], F16, kind="ExternalInput").ap()
    bv = nc.dram_tensor("bv", [1, DG], F16, kind="ExternalInput").ap()
    maskf = nc.dram_tensor("maskf", [128, KT], F32, kind="ExternalInput").ap()
    outp = nc.dram_tensor("outp", [L, D], F16, kind="ExternalOutput").ap()

    with tile.TileContext(nc) as tc:
        _emit(nc, tc, dict(xqT=xqT, xkT=xkT, xvT=xvT, aq=aq, ak=ak, av=av,
                           bo=bo, bq=bq, bk=bk, bv=bv, maskf=maskf,
                           outp=outp),
              use_bias, use_mask, q8, k8)
    nc.compile()
    _CACHE[key] = nc
    return nc


def _emit(nc, tc, t, use_bias, use_mask, q8, k8):
    from contextlib import ExitStack
    FQ = F8 if q8 else F16
    FK = F8 if k8 else F16
    # 1/sqrt(D), with the fp8 weight prescale(s) divided back out
    scale = 1.0 / 32.0
    if q8:
        scale /= W8SCALE
    if k8:
        scale /= W8SCALE
    ctx = ExitStack()
    with ctx:
        sb_w = ctx.enter_context(tc.tile_pool(name="sb_w", bufs=1))
        sb_qkv = ctx.enter_context(tc.tile_pool(name="sb_qkv", bufs=1))
        sb_pt = ctx.enter_context(tc.tile_pool(name="sb_pt", bufs=12))
        sb_n = ctx.enter_context(tc.tile_pool(name="sb_n", bufs=5))
        sb_out = ctx.enter_context(tc.tile_pool(name="sb_out", bufs=2))
        ps = ctx.enter_context(tc.tile_pool(name="ps", bufs=8, space="PSUM"))

        # ---- resident tiles ----
        aq_t = sb_w.tile([128, DC, DG], FQ, tag="aq")
        ak_t = sb_w.tile([128, DC, DG], FK, tag="ak")
        av_t = sb_w.tile([128, DC, DG], F16, tag="av")
        bo_t = sb_w.tile([128, 2, D], F16, tag="bo")
        ones_t = sb_w.tile([128, 512], F16, tag="ones")
        xq_res = sb_w.tile([128, DC, L], FQ, tag="xq")
        xk_res = sb_w.tile([128, DC, L], FK, tag="xk")
        xv_res = sb_w.tile([128, DC, L], F16, tag="xv")
        KT_sb = [sb_qkv.tile([128, L], F16, tag=f"kt{m}", name=f"KTm{m}")
                 for m in range(2)]
        QT_sb = [sb_qkv.tile([128, L], F16, tag=f"qt{h}", name=f"QTh{h}")
                 for h in range(GH)]
        V1 = sb_qkv.tile([128, KT, GH, HD + 1], F16, tag="v1")
        oN_sb = [sb_qkv.tile([128, 512], F16, tag=f"oN{hp}", name=f"oN{hp}")
                 for hp in range(2)]

        # ---- input DMAs: one priority-ordered queue (sync), in
        # consumption order: qproj(0) needs aq+xq0; then per column
        # group g: kproj(g) needs xk[:, g], vproj(g) needs xv[:, g].
        nc.sync.dma_start(out=aq_t, in_=t["aq"].rearrange("p (c d) -> p c d", c=DC))
        for c in range(DC):
            nc.sync.dma_start(
                out=xq_res[:, c, 0:512], in_=t["xqT"][c * 128:(c + 1) * 128, 0:512])
        nc.sync.dma_start(out=ak_t, in_=t["ak"].rearrange("p (c d) -> p c d", c=DC))
        if use_mask:
            mask_t = sb_w.tile([128, KT], F32, tag="mask")
            nc.sync.dma_start(out=mask_t, in_=t["maskf"])
        for g in range(4):
            for c in range(DC):
                nc.sync.dma_start(
                    out=xk_res[:, c, g * 512:(g + 1) * 512],
                    in_=t["xkT"][c * 128:(c + 1) * 128, g * 512:(g + 1) * 512])
            if g == 0:
                nc.sync.dma_start(
                    out=av_t, in_=t["av"].rearrange("p (c d) -> p c d", c=DC))
            for c in range(DC):
                nc.sync.dma_start(
                    out=xv_res[:, c, g * 512:(g + 1) * 512],
                    in_=t["xvT"][c * 128:(c + 1) * 128, g * 512:(g + 1) * 512])
        for qt in range(1, QT):
            for c in range(DC):
                nc.sync.dma_start(
                    out=xq_res[:, c, qt * 512:(qt + 1) * 512],
                    in_=t["xqT"][c * 128:(c + 1) * 128, qt * 512:(qt + 1) * 512])
        nc.sync.dma_start(out=bo_t, in_=t["bo"].rearrange("p (h d) -> p h d", h=2))
        bq_t = bk_t = bv_t = None
        if use_bias:
            bq_t = sb_w.tile([1, DG], F16, tag="bq")
            bk_t = sb_w.tile([1, DG], F16, tag="bk")
            bv_t = sb_w.tile([1, DG], F16, tag="bv")
            nc.sync.dma_start(out=bq_t, in_=t["bq"])
            nc.sync.dma_start(out=bk_t, in_=t["bk"])
            nc.sync.dma_start(out=bv_t, in_=t["bv"])

        # ones (bc stationary / bias paths) and V1 denominator column are
        # memset on DVE -- no DMA dependency, so attention never waits on
        # the tail of the input-DMA ring.
        nc.vector.memset(ones_t, 1.0)
        if not use_mask:
            nc.vector.memset(V1[:, :, :, HD:HD + 1], 1.0)

        # zero halves of per-head Q^T
        for h in range(GH):
            z0 = 0 if h % 2 else 64
            nc.vector.memset(QT_sb[h][z0:z0 + 64, :], 0.0)

        # ACT table warmup (Exp only; reciprocal runs on DVE)
        warm = sb_w.tile([1, 32], F32, tag="warm")
        nc.vector.memset(warm, 1.0)
        warm2 = sb_w.tile([1, 32], F32, tag="warm2")
        nc.scalar.activation(out=warm2, in_=warm, func=AF.Exp)

        # V1 ones column (masked path: scaled by the mask)
        if use_mask:
            ones4 = sb_w.tile([128, GH], F32, tag="ones4")
            nc.vector.memset(ones4, 1.0)
            for kt in range(KT):
                nc.vector.tensor_scalar_mul(
                    V1[:, kt, :, HD:HD + 1],
                    ones4.rearrange("p h -> p h 1"), mask_t[:, kt:kt + 1])

        # ---- emit helpers ----
        def emit_kproj(qt):
            """K proj for a 512-token column group."""
            c0 = qt * 512
            psm = [ps.tile([128, 512], F32, tag="o", bufs=2,
                           name=f"psk_{qt}_{_}") for _ in range(2)]
            if k8:
                for cp in range(DC // 2):
                    for m in range(2):
                        nc.tensor.matmul(
                            psm[m][:, 0:512],
                            ak_t[:, 2 * cp:2 * cp + 2, m * 128:(m + 1) * 128],
                            xk_res[:, 2 * cp:2 * cp + 2, c0:c0 + 512],
                            start=(cp == 0),
                            stop=(cp == DC // 2 - 1 and not use_bias),
                            perf_mode=DR)
            else:
                for c in range(DC):
                    xsl = xk_res[:, c, c0:c0 + 512]
                    for m in range(2):
                        nc.tensor.matmul(
                            psm[m][:, 0:512], ak_t[:, c, m * 128:(m + 1) * 128],
                            xsl, start=(c == 0),
                            stop=(c == DC - 1 and not use_bias))
            if use_bias:
                for m in range(2):
                    nc.tensor.matmul(
                        psm[m][:, 0:512], bk_t[:, m * 128:(m + 1) * 128],
                        ones_t[0:1, :], start=False, stop=True)
            for m in range(2):
                nc.vector.tensor_copy(
                    KT_sb[m][:, c0:c0 + 512], psm[m][:, 0:512])

        def emit_qproj(qt):
            """Q proj packed: one full-128 stationary per head pair."""
            psq = {hp: ps.tile([128, 512], F32, tag="o", bufs=2,
                               name=f"psq_{qt}_{hp}") for hp in range(2)}
            for hp in range(2):
                if q8:
                    for cp in range(DC // 2):
                        nc.tensor.matmul(
                            psq[hp][:, 0:512],
                            aq_t[:, 2 * cp:2 * cp + 2, hp * 128:(hp + 1) * 128],
                            xq_res[:, 2 * cp:2 * cp + 2, qt * 512:(qt + 1) * 512],
                            start=(cp == 0), stop=(cp == DC // 2 - 1 and not use_bias),
                            perf_mode=DR)
                else:
                    for c in range(DC):
                        nc.tensor.matmul(
                            psq[hp][:, 0:512],
                            aq_t[:, c, hp * 128:(hp + 1) * 128],
                            xq_res[:, c, qt * 512:(qt + 1) * 512],
                            start=(c == 0), stop=(c == DC - 1 and not use_bias))
                if use_bias:
                    nc.tensor.matmul(
                        psq[hp][:, 0:512], bq_t[:, hp * 128:(hp + 1) * 128],
                        ones_t[0:1, :], start=False, stop=True)
            for hp in range(2):
                for hh in range(2):
                    r0 = 64 * hh
                    nc.vector.tensor_copy(
                        QT_sb[2 * hp + hh][r0:r0 + 64, qt * 512:(qt + 1) * 512],
                        psq[hp][r0:r0 + 64, 0:512])

        def emit_vproj(ktg, jp):
            js = (jp * 2, jp * 2 + 1)
            psv = {j: ps.tile([128, 512], F32, tag="o", bufs=2,
                              name=f"psv_{ktg}_{j}") for j in js}
            for c in range(DC):
                xsl = xv_res[:, c, ktg * 512:(ktg + 1) * 512]
                for j in js:
                    nc.tensor.matmul(
                        psv[j][:, 0:DG], xsl[:, j * 128:(j + 1) * 128],
                        av_t[:, c, :],
                        start=(c == 0), stop=(c == DC - 1 and not use_bias))
            if use_bias:
                for j in js:
                    nc.tensor.matmul(
                        psv[j][:, 0:DG], ones_t[0:1, 0:128], bv_t,
                        start=False, stop=True)
            for j in js:
                kt = ktg * 4 + j
                srcv = psv[j][:, 0:DG].rearrange("p (h d) -> p h d", h=GH)
                if use_mask:
                    nc.vector.tensor_scalar_mul(
                        V1[:, kt, :, 0:HD], srcv, mask_t[:, kt:kt + 1])
                else:
                    nc.vector.tensor_copy(V1[:, kt, :, 0:HD], srcv)

        pts_store = {}
        oT_acc2 = [[sb_n.tile([65, 512], F32, tag=f"oTa{p}_{h}", bufs=1,
                              name=f"oTa{p}_{h}") for h in range(GH)]
                   for p in range(2)]

        def emit_s_half(qb, sk, hp, pss):
            """S^T for head pair hp of a 2-k-tile block, plus its exps."""
            qs0 = qb * QB
            for hh in range(2):
                h = hp * 2 + hh
                pss[h] = ps.tile([128, 1024], F32, tag="s", bufs=2,
                                 name=f"pss_{qb}_{sk}_{h}")
            for dk in range(2):
                kt = sk * 2 + dk
                for hh in range(2):
                    h = hp * 2 + hh
                    nc.tensor.matmul(
                        pss[h][:, dk * 512:(dk + 1) * 512],
                        KT_sb[hp][:, kt * 128:(kt + 1) * 128],
                        QT_sb[h][:, qs0:qs0 + QB],
                        start=True, stop=True)
            for hh in range(2):
                h = hp * 2 + hh
                pt = sb_pt.tile([128, 1024], F16, tag="pt",
                                name=f"pt_{qb}_{sk}_{h}")
                nc.scalar.activation(out=pt, in_=pss[h], func=AF.Exp,
                                     scale=scale)
                pts_store[(sk, h)] = pt

        def emit_pv_qh(qb, q, h):
            """PV for k-tiles 4q..4q+3, one head: short psum chain
            accumulated into SBUF oT_acc."""
            oT_acc = oT_acc2[qb % 2]
            po = ps.tile([128, 512], F32, tag="a", bufs=2,
                         name=f"po_{qb}_{q}_{h}")
            for kt in range(4 * q, 4 * q + 4):
                nc.tensor.matmul(
                    po[0:HD + 1, :], V1[:, kt, h, :],
                    pts_store[(kt // 2, h)][:, (kt % 2) * 512:(kt % 2) * 512 + 512],
                    start=(kt % 4 == 0), stop=(kt % 4 == 3))
            if q == 0:
                nc.vector.tensor_copy(oT_acc[h], po[0:65, :])
            else:
                nc.vector.tensor_tensor(oT_acc[h], oT_acc[h],
                                        po[0:65, :], op=ADD)

        def emit_norm_hp(qb, hp):
            """Reciprocal of denominators, broadcast, normalize one packed
            oN pair."""
            oT_acc = oT_acc2[qb % 2]
            bc = ps.tile([128, 512], F32, tag="o", bufs=2,
                         name=f"bc_{qb}_{hp}")
            oN = oN_sb[hp]
            for hh in range(2):
                h = 2 * hp + hh
                oT = oT_acc[h]
                # 1/denominator on DVE (18-bit approx), keeping ScalarE
                # free for the softmax exps; f16 cast for the broadcast mm
                rr32 = sb_n.tile([65, 512], F32, tag="rr32",
                                 name=f"rr32_{qb}_{h}")
                nc.vector.reciprocal(out=rr32[64:65, :], in_=oT[64:65, :])
                rr = sb_n.tile([65, 512], F16, tag="rr", name=f"rr_{qb}_{h}")
                nc.vector.tensor_copy(rr[64:65, :], rr32[64:65, :])
                nc.tensor.matmul(
                    bc[64 * hh:64 * hh + 64, :], ones_t[64:65, 0:64],
                    rr[64:65, :], start=True, stop=True,
                    tile_position=(64, 64 * hh))
            for hh in range(2):
                h = 2 * hp + hh
                nc.vector.tensor_tensor(
                    oN[64 * hh:64 * hh + 64, :],
                    oT_acc[h][0:64, :],
                    bc[64 * hh:64 * hh + 64, :], op=MULT)

        def emit_out_mq(qb, mq):
            psout = [ps.tile([128, 512], F32, tag="o", bufs=2,
                             name=f"psout_{qb}_{mq}_{_}") for _ in range(2)]
            for hp in range(2):
                for nb in range(2):
                    nc.tensor.matmul(
                        psout[nb][:, 0:512],
                        oN_sb[hp][:, mq * 128:(mq + 1) * 128],
                        bo_t[:, hp, nb * 512:(nb + 1) * 512],
                        start=(hp == 0), stop=(hp == 1))
            ot = sb_out.tile([128, D], F16, tag="ot", name=f"ot_{qb}_{mq}")
            for nb in range(2):
                nc.vector.tensor_copy(ot[:, nb * 512:(nb + 1) * 512],
                                      psout[nb][:, 0:512])
            q0 = qb * QB + mq * 128
            nc.gpsimd.dma_start(out=t["outp"][q0:q0 + 128, :], in_=ot)

        # ---- schedule: projections (k/v interleaved with DMA arrival);
        # then per qb: S head-pair halves with a woven filler queue of
        # TensorE work (PV chains, next qproj, previous norm/out-proj) so
        # the engine never waits on the S->exp round trip.  fillers[i] is
        # the list of thunks emitted after half-slot i (2 per sk).
        emit_qproj(0)
        for g in range(4):
            emit_kproj(g)
            emit_vproj(g, 0)
            emit_vproj(g, 1)
        for qb in range(QT):
            F = [[] for _ in range(16)]
            if qb >= 1:
                # PV tail quarter of the previous qb, then its norm + out
                for h in range(GH):
                    F[h // 2].append(
                        lambda h=h: emit_pv_qh(qb - 1, 3, h))
                F[3].append(lambda: emit_norm_hp(qb - 1, 0))
                F[4].append(lambda: emit_norm_hp(qb - 1, 1))
                for mq, sl in ((0, 6), (1, 7), (2, 9), (3, 10)):
                    F[sl].append(
                        lambda mq=mq: emit_out_mq(qb - 1, mq))
            if qb + 1 < QT:
                F[2].append(lambda: emit_qproj(qb + 1))
            # this qb's PV quarters 0..2 (quarter q needs exps of sk 2q+1,
            # emitted by half-slot 4q+3)
            for q in range(3):
                for h, sl in ((0, 0), (1, 1), (2, 1), (3, 2)):
                    F[4 * q + 4 + sl].append(
                        lambda q=q, h=h: emit_pv_qh(qb, q, h))
            pss = {}
            for sk in range(KT // 2):
                for hp in range(2):
                    emit_s_half(qb, sk, hp, pss)
                    for f in F[2 * sk + hp]:
                        f()
        # final drain: last qb's PV tail quarter, norm, out-projection
        for h in range(GH):
            emit_pv_qh(QT - 1, 3, h)
        emit_norm_hp(QT - 1, 0)
        emit_norm_hp(QT - 1, 1)
        for mq in range(4):
            emit_out_mq(QT - 1, mq)


def _swizzle_a(aT):
    """[D, DG] -> [128, DC*DG]: partition p holds chunks c at (c, :)."""
    return np.ascontiguousarray(
        aT.reshape(DC, 128, DG).transpose(1, 0, 2).reshape(128, DC * DG))


def _f8(x):
    return np.clip(x, -240.0, 240.0).astype(ml_dtypes.float8_e4m3)


def _prep_inputs(values, key, query, mask, Wv, Wk, Wq, Wo, bv, bk, bq):
    """Build the 8 per-core input maps (host-side shard + layout)."""
    q8 = FP8_MODE == "qk"
    k8 = FP8_MODE in ("qk", "k")
    xT = {}
    for n in range(N_BATCH):
        qT = np.ascontiguousarray(query[n].T)
        kTr = np.ascontiguousarray(key[n].T)
        xT[("q", n)] = _f8(qT) if q8 else qT.astype(np.float16)
        xT[("k", n)] = _f8(kTr) if k8 else kTr.astype(np.float16)
        xT[("v", n)] = np.ascontiguousarray(values[n].T.astype(np.float16))
    in_maps = []
    for c in range(CORES):
        n, g = divmod(c, CORES // N_BATCH)
        rows = slice(g * DG, (g + 1) * DG)
        mrow = np.ascontiguousarray(
            mask[n, 0, 0, :].astype(np.float32).reshape(KT, 128).T)
        aq_h = Wq[rows, :].T
        ak_h = Wk[rows, :].T
        aq_c = _swizzle_a(_f8(aq_h * W8SCALE)) if q8 else \
            _swizzle_a(aq_h.astype(np.float16))
        ak_c = _swizzle_a(_f8(ak_h * W8SCALE)) if k8 else \
            _swizzle_a(ak_h.astype(np.float16))
        # bo: [128, 2, D]; partition r of pair hp = Wo row for local dim
        # (g*DG + hp*128 + r)
        bo_c = np.ascontiguousarray(
            Wo[:, g * DG:(g + 1) * DG].T.astype(np.float16).reshape(
                2, 128, D).transpose(1, 0, 2).reshape(128, 2 * D))
        in_maps.append({
            "xqT": xT[("q", n)],
            "xkT": xT[("k", n)],
            "xvT": xT[("v", n)],
            "aq": aq_c,
            "ak": ak_c,
            "av": _swizzle_a(Wv[rows, :].T.astype(np.float16)),
            "bo": bo_c,
            "bq": np.ascontiguousarray(bq[None, rows].astype(np.float16)),
            "bk": np.ascontiguousarray(bk[None, rows].astype(np.float16)),
            "bv": np.ascontiguousarray(bv[None, rows].astype(np.float16)),
            "maskf": mrow,
        })
    return in_maps


LAST_EXEC_NS = None


def kernel(values, key, query, mask, Wv, bv, Wk, bk, Wq, bq, Wo, bo,
           trace=False):
    global LAST_EXEC_NS
    values = np.asarray(values, dtype=np.float32)
    key = np.asarray(key, dtype=np.float32)
    query = np.asarray(query, dtype=np.float32)
    mask = np.asarray(mask)
    Wq, Wk, Wv, Wo = (np.asarray(Wq, np.float32), np.asarray(Wk, np.float32),
                      np.asarray(Wv, np.float32), np.asarray(Wo, np.float32))
    bq, bk, bv, bo = (np.asarray(bq, np.float32), np.asarray(bk, np.float32),
                      np.asarray(bv, np.float32), np.asarray(bo, np.float32))

    use_bias = bool(np.any(bq) or np.any(bk) or np.any(bv))
    use_mask = not bool(np.all(np.asarray(mask) == 1))

    nc = _build(use_bias, use_mask)
    in_maps = _prep_inputs(values, key, query, mask, Wv, Wk, Wq, Wo,
                           bv, bk, bq)
    res = run_bass_kernel_spmd(nc, in_maps, core_ids=list(range(CORES)),
                               trace=trace)
    LAST_EXEC_NS = res.exec_time_ns

    out = np.zeros((N_BATCH, L, D), dtype=np.float32)
    for c in range(CORES):
        n = c // (CORES // N_BATCH)
        out[n] += res.results[c]["outp"].astype(np.float32)
    out += bo[None, None, :]
    return out


# revision 22
# speedup vs baseline: 1.2019x; 1.2019x over previous
"""Multi-head attention (N=2, L=2048, D=1024, H=16) on 8 NeuronCores.

Sharding: core c -> (batch n = c // 4, head group g = c % 4, 4 heads each).
Each core computes Q/K/V projections for its 4 heads, flash-style attention
(S^T = K @ Q^T per k-tile, exp on ScalarE with 1/sqrt(D) folded into the
activation scale, P^T @ V via TensorE with a ones-column appended to V to get
the softmax denominator for free), normalization, and its slice of the output
projection. Host sums the 4 partial output projections per batch and adds bo.

Matmul packing: Q-projection computes head PAIRS with a full 128-row
stationary; the output projection contracts packed head-pair oN tiles
against [128, D] Wo slices (full contraction). K-projection uses
1024-wide moving operands on the wide PSUM slots. Optional fp8e4
DoubleRow path for the Q/K projections (FP8_MODE=qk|k).
"""
import os
import sys
import types

import numpy as np
import ml_dtypes

N_BATCH = 2
L = 2048
D = 1024
H = 16
HD = 64
CORES = 8
GH = 4            # heads per core
DG = GH * HD      # 256 = projected dims per core
QB = 512          # q block
KT = L // 128     # 16 k tiles
QT = L // QB      # 4 q blocks
DC = D // 128     # 8 din chunks
FP8_MODE = os.environ.get("FP8_MODE", "0")   # "0" | "k" | "qk"
W8SCALE = 16.0    # fp8 weight prescale


def _install_ntff_hook():
    """The image's antenv stub lacks axon_hooks; shim it so trace=True works."""
    if "antenv.axon_hooks" in sys.modules:
        return
    mod = types.ModuleType("antenv.axon_hooks")
    mod._hook = None
    mod.set_axon_ntff_profile_hook = lambda h: setattr(mod, "_hook", h)
    mod.get_axon_ntff_profile_hook = lambda: mod._hook
    sys.modules["antenv.axon_hooks"] = mod
    try:
        from trn_agent_boot.trn_boot import _ntff_profile_via_ctypes
        mod._hook = _ntff_profile_via_ctypes("/opt/axon/libaxon_pjrt.so")
    except Exception:
        mod._hook = None


_install_ntff_hook()

import concourse.bacc as bacc
import concourse.mybir as mybir
import concourse.tile as tile
from concourse.bass_utils import run_bass_kernel_spmd

F32 = mybir.dt.float32
F16 = mybir.dt.float16
F8 = mybir.dt.float8e4
AF = mybir.ActivationFunctionType
MULT = mybir.AluOpType.mult
ADD = mybir.AluOpType.add
DR = mybir.MatmulPerfMode.DoubleRow

_CACHE = {}


_TABLES_PATCHED = False


def _patch_act_tables():
    """Prefer natural_log_exp_and_others so Exp and Ln share one table set."""
    global _TABLES_PATCHED
    if _TABLES_PATCHED:
        return
    import concourse.bacc as _bacc
    import concourse.hw_specs as _hw
    orig_fn = _hw.get_activation_tables

    def patched(arch):
        import concourse.mybir as _mybir
        tabs = dict(orig_fn(arch))
        pref = "natural_log_exp_and_others"
        if pref not in tabs:
            return tabs
        drop = {_mybir.ActivationFunctionType.Exp,
                _mybir.ActivationFunctionType.Ln}
        return {k: (v if k == pref else (set(v) - drop))
                for k, v in tabs.items()}

    _bacc.get_activation_tables = patched
    _TABLES_PATCHED = True


def _build(use_bias, use_mask):
    key = (use_bias, use_mask, FP8_MODE)
    if key in _CACHE:
        return _CACHE[key]
    if os.environ.get("ACT_TABLE_PATCH", "1") == "1":
        _patch_act_tables()

    nc = bacc.Bacc("TRN2", debug=False, num_devices=CORES)

    q8 = FP8_MODE == "qk"
    k8 = FP8_MODE in ("qk", "k")
    FQ = F8 if q8 else F16
    FK = F8 if k8 else F16

    xqT = nc.dram_tensor("xqT", [D, L], FQ, kind="ExternalInput").ap()
    xkT = nc.dram_tensor("xkT", [D, L], FK, kind="ExternalInput").ap()
    xvT = nc.dram_tensor("xvT", [D, L], F16, kind="ExternalInput").ap()
    aq = nc.dram_tensor("aq", [128, DC * DG], FQ, kind="ExternalInput").ap()
    ak = nc.dram_tensor("ak", [128, DC * DG], FK, kind="ExternalInput").ap()
    av = nc.dram_tensor("av", [128, DC * DG], F16, kind="ExternalInput").ap()
    bo = nc.dram_tensor("bo", [128, 2 * D], F16, kind="ExternalInput").ap()
    bq = nc.dram_tensor("bq", [1, DG], F16, kind="ExternalInput").ap()
    bk = nc.dram_tensor("bk", [1, DG], F16, kind="ExternalInput").ap()
    bv = nc.dram_tensor("bv", [1, DG], F16, kind="ExternalInput").ap()
    maskf = nc.dram_tensor("maskf", [128, KT], F32, kind="ExternalInput").ap()
    outp = nc.dram_tensor("outp", [L, D], F16, kind="ExternalOutput").ap()

    with tile.TileContext(nc) as tc:
        _emit(nc, tc, dict(xqT=xqT, xkT=xkT, xvT=xvT, aq=aq, ak=ak, av=av,
                           bo=bo, bq=bq, bk=bk, bv=bv, maskf=maskf,
                           outp=outp),
              use_bias, use_mask, q8, k8)
    nc.compile()
    _CACHE[key] = nc
    return nc


def _emit(nc, tc, t, use_bias, use_mask, q8, k8):
    from contextlib import ExitStack
    FQ = F8 if q8 else F16
    FK = F8 if k8 else F16
    # 1/sqrt(D), with the fp8 weight prescale(s) divided back out
    scale = 1.0 / 32.0
    if q8:
        scale /= W8SCALE
    if k8:
        scale /= W8SCALE
    ctx = ExitStack()
    with ctx:
        sb_w = ctx.enter_context(tc.tile_pool(name="sb_w", bufs=1))
        sb_qkv = ctx.enter_context(tc.tile_pool(name="sb_qkv", bufs=1))
        sb_pt = ctx.enter_context(tc.tile_pool(name="sb_pt", bufs=12))
        sb_n = ctx.enter_context(tc.tile_pool(name="sb_n", bufs=5))
        sb_out = ctx.enter_context(tc.tile_pool(name="sb_out", bufs=2))
        ps = ctx.enter_context(tc.tile_pool(name="ps", bufs=8, space="PSUM"))

        # ---- resident tiles ----
        aq_t = sb_w.tile([128, DC, DG], FQ, tag="aq")
        ak_t = sb_w.tile([128, DC, DG], FK, tag="ak")
        av_t = sb_w.tile([128, DC, DG], F16, tag="av")
        bo_t = sb_w.tile([128, 2, D], F16, tag="bo")
        ones_t = sb_w.tile([128, 512], F16, tag="ones")
        xq_res = sb_w.tile([128, DC, L], FQ, tag="xq")
        xk_res = sb_w.tile([128, DC, L], FK, tag="xk")
        xv_res = sb_w.tile([128, DC, L], F16, tag="xv")
        KT_sb = [sb_qkv.tile([128, L], F16, tag=f"kt{m}", name=f"KTm{m}")
                 for m in range(2)]
        QT_sb = [sb_qkv.tile([128, L], F16, tag=f"qt{h}", name=f"QTh{h}")
                 for h in range(GH)]
        V1 = sb_qkv.tile([128, KT, GH, HD + 1], F16, tag="v1")
        oN_sb = [sb_qkv.tile([128, 512], F16, tag=f"oN{hp}", name=f"oN{hp}")
                 for hp in range(2)]

        # ---- input DMAs: one priority-ordered queue (sync), in
        # consumption order: qproj(0) needs aq+xq0; then per column
        # group g: kproj(g) needs xk[:, g], vproj(g) needs xv[:, g].
        nc.sync.dma_start(out=aq_t, in_=t["aq"].rearrange("p (c d) -> p c d", c=DC))
        for c in range(DC):
            nc.sync.dma_start(
                out=xq_res[:, c, 0:512], in_=t["xqT"][c * 128:(c + 1) * 128, 0:512])
        nc.sync.dma_start(out=ak_t, in_=t["ak"].rearrange("p (c d) -> p c d", c=DC))
        if use_mask:
            mask_t = sb_w.tile([128, KT], F32, tag="mask")
            nc.sync.dma_start(out=mask_t, in_=t["maskf"])
        for g in range(4):
            for c in range(DC):
                nc.sync.dma_start(
                    out=xk_res[:, c, g * 512:(g + 1) * 512],
                    in_=t["xkT"][c * 128:(c + 1) * 128, g * 512:(g + 1) * 512])
            if g == 0:
                nc.sync.dma_start(
                    out=av_t, in_=t["av"].rearrange("p (c d) -> p c d", c=DC))
            for c in range(DC):
                nc.sync.dma_start(
                    out=xv_res[:, c, g * 512:(g + 1) * 512],
                    in_=t["xvT"][c * 128:(c + 1) * 128, g * 512:(g + 1) * 512])
        for qt in range(1, QT):
            for c in range(DC):
                nc.sync.dma_start(
                    out=xq_res[:, c, qt * 512:(qt + 1) * 512],
                    in_=t["xqT"][c * 128:(c + 1) * 128, qt * 512:(qt + 1) * 512])
        nc.sync.dma_start(out=bo_t, in_=t["bo"].rearrange("p (h d) -> p h d", h=2))
        bq_t = bk_t = bv_t = None
        if use_bias:
            bq_t = sb_w.tile([1, DG], F16, tag="bq")
            bk_t = sb_w.tile([1, DG], F16, tag="bk")
            bv_t = sb_w.tile([1, DG], F16, tag="bv")
            nc.sync.dma_start(out=bq_t, in_=t["bq"])
            nc.sync.dma_start(out=bk_t, in_=t["bk"])
            nc.sync.dma_start(out=bv_t, in_=t["bv"])

        # ones (bc stationary / bias paths) and V1 denominator column are
        # memset on DVE -- no DMA dependency, so attention never waits on
        # the tail of the input-DMA ring.
        nc.vector.memset(ones_t, 1.0)
        if not use_mask:
            nc.vector.memset(V1[:, :, :, HD:HD + 1], 1.0)

        # zero halves of per-head Q^T
        for h in range(GH):
            z0 = 0 if h % 2 else 64
            nc.vector.memset(QT_sb[h][z0:z0 + 64, :], 0.0)

        # ACT table warmup (Exp only; reciprocal runs on DVE)
        warm = sb_w.tile([1, 32], F32, tag="warm")
        nc.vector.memset(warm, 1.0)
        warm2 = sb_w.tile([1, 32], F32, tag="warm2")
        nc.scalar.activation(out=warm2, in_=warm, func=AF.Exp)

        # V1 ones column (masked path: scaled by the mask)
        if use_mask:
            ones4 = sb_w.tile([128, GH], F32, tag="ones4")
            nc.vector.memset(ones4, 1.0)
            for kt in range(KT):
                nc.vector.tensor_scalar_mul(
                    V1[:, kt, :, HD:HD + 1],
                    ones4.rearrange("p h -> p h 1"), mask_t[:, kt:kt + 1])

        # ---- emit helpers ----
        def emit_kproj(qt):
            """K proj for a 512-token column group."""
            c0 = qt * 512
            psm = [ps.tile([128, 512], F32, tag="o", bufs=2,
                           name=f"psk_{qt}_{_}") for _ in range(2)]
            if k8:
                for cp in range(DC // 2):
                    for m in range(2):
                        nc.tensor.matmul(
                            psm[m][:, 0:512],
                            ak_t[:, 2 * cp:2 * cp + 2, m * 128:(m + 1) * 128],
                            xk_res[:, 2 * cp:2 * cp + 2, c0:c0 + 512],
                            start=(cp == 0),
                            stop=(cp == DC // 2 - 1 and not use_bias),
                            perf_mode=DR)
            else:
                for c in range(DC):
                    xsl = xk_res[:, c, c0:c0 + 512]
                    for m in range(2):
                        nc.tensor.matmul(
                            psm[m][:, 0:512], ak_t[:, c, m * 128:(m + 1) * 128],
                            xsl, start=(c == 0),
                            stop=(c == DC - 1 and not use_bias))
            if use_bias:
                for m in range(2):
                    nc.tensor.matmul(
                        psm[m][:, 0:512], bk_t[:, m * 128:(m + 1) * 128],
                        ones_t[0:1, :], start=False, stop=True)
            for m in range(2):
                nc.vector.tensor_copy(
                    KT_sb[m][:, c0:c0 + 512], psm[m][:, 0:512])

        def emit_qproj(qt):
            """Q proj packed: one full-128 stationary per head pair."""
            psq = {hp: ps.tile([128, 512], F32, tag="o", bufs=2,
                               name=f"psq_{qt}_{hp}") for hp in range(2)}
            for hp in range(2):
                if q8:
                    for cp in range(DC // 2):
                        nc.tensor.matmul(
                            psq[hp][:, 0:512],
                            aq_t[:, 2 * cp:2 * cp + 2, hp * 128:(hp + 1) * 128],
                            xq_res[:, 2 * cp:2 * cp + 2, qt * 512:(qt + 1) * 512],
                            start=(cp == 0), stop=(cp == DC // 2 - 1 and not use_bias),
                            perf_mode=DR)
                else:
                    for c in range(DC):
                        nc.tensor.matmul(
                            psq[hp][:, 0:512],
                            aq_t[:, c, hp * 128:(hp + 1) * 128],
                            xq_res[:, c, qt * 512:(qt + 1) * 512],
                            start=(c == 0), stop=(c == DC - 1 and not use_bias))
                if use_bias:
                    nc.tensor.matmul(
                        psq[hp][:, 0:512], bq_t[:, hp * 128:(hp + 1) * 128],
                        ones_t[0:1, :], start=False, stop=True)
            for hp in range(2):
                for hh in range(2):
                    r0 = 64 * hh
                    nc.vector.tensor_copy(
                        QT_sb[2 * hp + hh][r0:r0 + 64, qt * 512:(qt + 1) * 512],
                        psq[hp][r0:r0 + 64, 0:512])

        def emit_vproj(ktg, jp):
            js = (jp * 2, jp * 2 + 1)
            psv = {j: ps.tile([128, 512], F32, tag="o", bufs=2,
                              name=f"psv_{ktg}_{j}") for j in js}
            for c in range(DC):
                xsl = xv_res[:, c, ktg * 512:(ktg + 1) * 512]
                for j in js:
                    nc.tensor.matmul(
                        psv[j][:, 0:DG], xsl[:, j * 128:(j + 1) * 128],
                        av_t[:, c, :],
                        start=(c == 0), stop=(c == DC - 1 and not use_bias))
            if use_bias:
                for j in js:
                    nc.tensor.matmul(
                        psv[j][:, 0:DG], ones_t[0:1, 0:128], bv_t,
                        start=False, stop=True)
            for j in js:
                kt = ktg * 4 + j
                srcv = psv[j][:, 0:DG].rearrange("p (h d) -> p h d", h=GH)
                if use_mask:
                    nc.vector.tensor_scalar_mul(
                        V1[:, kt, :, 0:HD], srcv, mask_t[:, kt:kt + 1])
                else:
                    nc.vector.tensor_copy(V1[:, kt, :, 0:HD], srcv)

        pts_store = {}
        rr_store = {}
        oT_acc2 = [sb_n.tile([65, GH, 512], F32, tag=f"oTa{p}", bufs=1,
                             name=f"oTa{p}") for p in range(2)]

        def emit_s_half(qb, sk, hp, pss):
            """S^T for head pair hp of a 2-k-tile block, plus its exps."""
            qs0 = qb * QB
            for hh in range(2):
                h = hp * 2 + hh
                pss[h] = ps.tile([128, 1024], F32, tag="s", bufs=2,
                                 name=f"pss_{qb}_{sk}_{h}")
            for dk in range(2):
                kt = sk * 2 + dk
                for hh in range(2):
                    h = hp * 2 + hh
                    nc.tensor.matmul(
                        pss[h][:, dk * 512:(dk + 1) * 512],
                        KT_sb[hp][:, kt * 128:(kt + 1) * 128],
                        QT_sb[h][:, qs0:qs0 + QB],
                        start=True, stop=True)
            for hh in range(2):
                h = hp * 2 + hh
                pt = sb_pt.tile([128, 1024], F16, tag="pt",
                                name=f"pt_{qb}_{sk}_{h}")
                nc.scalar.activation(out=pt, in_=pss[h], func=AF.Exp,
                                     scale=scale)
                pts_store[(sk, h)] = pt

        def emit_pv_qh(qb, q, h):
            """PV for k-tiles 4q..4q+3, one head: short psum chain
            accumulated into SBUF oT_acc."""
            oT_acc = oT_acc2[qb % 2]
            po = ps.tile([128, 512], F32, tag="a", bufs=2,
                         name=f"po_{qb}_{q}_{h}")
            for kt in range(4 * q, 4 * q + 4):
                nc.tensor.matmul(
                    po[0:HD + 1, :], V1[:, kt, h, :],
                    pts_store[(kt // 2, h)][:, (kt % 2) * 512:(kt % 2) * 512 + 512],
                    start=(kt % 4 == 0), stop=(kt % 4 == 3))
            if q == 0:
                nc.vector.tensor_copy(oT_acc[:, h, :], po[0:65, :])
            else:
                nc.vector.tensor_tensor(oT_acc[:, h, :], oT_acc[:, h, :],
                                        po[0:65, :], op=ADD)

        def emit_norm_hp(qb, hp):
            """Reciprocal of denominators, broadcast, normalize one packed
            oN pair."""
            oT_acc = oT_acc2[qb % 2]
            bc = ps.tile([128, 512], F32, tag="o", bufs=2,
                         name=f"bc_{qb}_{hp}")
            oN = oN_sb[hp]
            if hp == 0:
                # one batched Ln + negated-Exp over all 4 heads' denominator
                # rows gives 1/d with a single pair of ScalarE ops per qb
                lnr = sb_n.tile([65, GH, 512], F32, tag="lnr", bufs=1,
                                name=f"lnr_{qb}")
                nc.scalar.activation(out=lnr[64:65, :, :],
                                     in_=oT_acc[64:65, :, :], func=AF.Ln)
                rr = sb_n.tile([65, GH, 512], F16, tag="rr", bufs=2,
                               name=f"rr_{qb}")
                nc.scalar.activation(out=rr[64:65, :, :],
                                     in_=lnr[64:65, :, :],
                                     func=AF.Exp, scale=-1.0)
                rr_store[qb] = rr
            rr = rr_store[qb]
            for hh in range(2):
                h = 2 * hp + hh
                nc.tensor.matmul(
                    bc[64 * hh:64 * hh + 64, :], ones_t[64:65, 0:64],
                    rr[64:65, h, :], start=True, stop=True,
                    tile_position=(64, 64 * hh))
            for hh in range(2):
                h = 2 * hp + hh
                nc.vector.tensor_tensor(
                    oN[64 * hh:64 * hh + 64, :],
                    oT_acc[0:64, h, :],
                    bc[64 * hh:64 * hh + 64, :], op=MULT)

        def emit_out_mq(qb, mq):
            psout = [ps.tile([128, 512], F32, tag="o", bufs=2,
                             name=f"psout_{qb}_{mq}_{_}") for _ in range(2)]
            for hp in range(2):
                for nb in range(2):
                    nc.tensor.matmul(
                        psout[nb][:, 0:512],
                        oN_sb[hp][:, mq * 128:(mq + 1) * 128],
                        bo_t[:, hp, nb * 512:(nb + 1) * 512],
                        start=(hp == 0), stop=(hp == 1))
            ot = sb_out.tile([128, D], F16, tag="ot", name=f"ot_{qb}_{mq}")
            for nb in range(2):
                nc.vector.tensor_copy(ot[:, nb * 512:(nb + 1) * 512],
                                      psout[nb][:, 0:512])
            q0 = qb * QB + mq * 128
            nc.gpsimd.dma_start(out=t["outp"][q0:q0 + 128, :], in_=ot)

        # ---- schedule: projections (k/v interleaved with DMA arrival);
        # then per qb: S head-pair halves with a woven filler queue of
        # TensorE work (PV chains, next qproj, previous norm/out-proj) so
        # the engine never waits on the S->exp round trip.  fillers[i] is
        # the list of thunks emitted after half-slot i (2 per sk).
        emit_qproj(0)
        for g in range(4):
            emit_kproj(g)
            emit_vproj(g, 0)
            emit_vproj(g, 1)
        for qb in range(QT):
            F = [[] for _ in range(16)]
            if qb >= 1:
                # PV tail quarter of the previous qb, then its norm + out
                for h in range(GH):
                    F[h // 2].append(
                        lambda h=h: emit_pv_qh(qb - 1, 3, h))
                F[3].append(lambda: emit_norm_hp(qb - 1, 0))
                F[4].append(lambda: emit_norm_hp(qb - 1, 1))
                for mq, sl in ((0, 6), (1, 7), (2, 9), (3, 10)):
                    F[sl].append(
                        lambda mq=mq: emit_out_mq(qb - 1, mq))
            if qb + 1 < QT:
                F[2].append(lambda: emit_qproj(qb + 1))
            # this qb's PV quarters 0..2 (quarter q needs exps of sk 2q+1,
            # emitted by half-slot 4q+3)
            for q in range(3):
                for h, sl in ((0, 0), (1, 1), (2, 1), (3, 2)):
                    F[4 * q + 4 + sl].append(
                        lambda q=q, h=h: emit_pv_qh(qb, q, h))
            pss = {}
            for sk in range(KT // 2):
                for hp in range(2):
                    emit_s_half(qb, sk, hp, pss)
                    for f in F[2 * sk + hp]:
                        f()
        # final drain: last qb's PV tail quarter, norm, out-projection
        for h in range(GH):
            emit_pv_qh(QT - 1, 3, h)
        emit_norm_hp(QT - 1, 0)
        emit_norm_hp(QT - 1, 1)
        for mq in range(4):
            emit_out_mq(QT - 1, mq)


def _swizzle_a(aT):
    """[D, DG] -> [128, DC*DG]: partition p holds chunks c at (c, :)."""
    return np.ascontiguousarray(
        aT.reshape(DC, 128, DG).transpose(1, 0, 2).reshape(128, DC * DG))


def _f8(x):
    return np.clip(x, -240.0, 240.0).astype(ml_dtypes.float8_e4m3)


def _prep_inputs(values, key, query, mask, Wv, Wk, Wq, Wo, bv, bk, bq):
    """Build the 8 per-core input maps (host-side shard + layout)."""
    q8 = FP8_MODE == "qk"
    k8 = FP8_MODE in ("qk", "k")
    xT = {}
    for n in range(N_BATCH):
        qT = np.ascontiguousarray(query[n].T)
        kTr = np.ascontiguousarray(key[n].T)
        xT[("q", n)] = _f8(qT) if q8 else qT.astype(np.float16)
        xT[("k", n)] = _f8(kTr) if k8 else kTr.astype(np.float16)
        xT[("v", n)] = np.ascontiguousarray(values[n].T.astype(np.float16))
    in_maps = []
    for c in range(CORES):
        n, g = divmod(c, CORES // N_BATCH)
        rows = slice(g * DG, (g + 1) * DG)
        mrow = np.ascontiguousarray(
            mask[n, 0, 0, :].astype(np.float32).reshape(KT, 128).T)
        aq_h = Wq[rows, :].T
        ak_h = Wk[rows, :].T
        aq_c = _swizzle_a(_f8(aq_h * W8SCALE)) if q8 else \
            _swizzle_a(aq_h.astype(np.float16))
        ak_c = _swizzle_a(_f8(ak_h * W8SCALE)) if k8 else \
            _swizzle_a(ak_h.astype(np.float16))
        # bo: [128, 2, D]; partition r of pair hp = Wo row for local dim
        # (g*DG + hp*128 + r)
        bo_c = np.ascontiguousarray(
            Wo[:, g * DG:(g + 1) * DG].T.astype(np.float16).reshape(
                2, 128, D).transpose(1, 0, 2).reshape(128, 2 * D))
        in_maps.append({
            "xqT": xT[("q", n)],
            "xkT": xT[("k", n)],
            "xvT": xT[("v", n)],
            "aq": aq_c,
            "ak": ak_c,
            "av": _swizzle_a(Wv[rows, :].T.astype(np.float16)),
            "bo": bo_c,
            "bq": np.ascontiguousarray(bq[None, rows].astype(np.float16)),
            "bk": np.ascontiguousarray(bk[None, rows].astype(np.float16)),
            "bv": np.ascontiguousarray(bv[None, rows].astype(np.float16)),
            "maskf": mrow,
        })
    return in_maps


LAST_EXEC_NS = None


def kernel(values, key, query, mask, Wv, bv, Wk, bk, Wq, bq, Wo, bo,
           trace=False):
    global LAST_EXEC_NS
    values = np.asarray(values, dtype=np.float32)
    key = np.asarray(key, dtype=np.float32)
    query = np.asarray(query, dtype=np.float32)
    mask = np.asarray(mask)
    Wq, Wk, Wv, Wo = (np.asarray(Wq, np.float32), np.asarray(Wk, np.float32),
                      np.asarray(Wv, np.float32), np.asarray(Wo, np.float32))
    bq, bk, bv, bo = (np.asarray(bq, np.float32), np.asarray(bk, np.float32),
                      np.asarray(bv, np.float32), np.asarray(bo, np.float32))

    use_bias = bool(np.any(bq) or np.any(bk) or np.any(bv))
    use_mask = not bool(np.all(np.asarray(mask) == 1))

    nc = _build(use_bias, use_mask)
    in_maps = _prep_inputs(values, key, query, mask, Wv, Wk, Wq, Wo,
                           bv, bk, bq)
    res = run_bass_kernel_spmd(nc, in_maps, core_ids=list(range(CORES)),
                               trace=trace)
    LAST_EXEC_NS = res.exec_time_ns

    out = np.zeros((N_BATCH, L, D), dtype=np.float32)
    for c in range(CORES):
        n = c // (CORES // N_BATCH)
        out[n] += res.results[c]["outp"].astype(np.float32)
    out += bo[None, None, :]
    return out
